# revision 1
# baseline (speedup 1.0000x reference)
"""Causal self-attention (B=4, T=2048, C=1024, H=16, D=64) on 8 TRN2 NeuronCores.

Sharding: 8 cores = 4 batches x 2 head-groups (8 heads each). Each core:
  - QKV projection for its (batch, head-group) column slice of w_attn,
    producing qT/kT in [d, t] layout (transposed dataflow) and v in [t, d].
  - Causal attention in scoresT layout (scores^T[k, q] comes straight out of
    the PE; softmax denominators via an appended ones-column on V; no PE
    transposes anywhere).
  - Row-sharded output projection -> per-core partial [T, C].
Host sums the two partials per batch and adds b_proj.

Matmul operands are bf16 (1 cycle/row on the PE) with all accumulation in
fp32 PSUM. The three phases are software-pipelined: attention on the first
half of the sequence (q < 1024) only needs projection quarters 0-1, so
quarters 2-3 interleave with it (the PE projects while ACT runs exp);
the first half of the output projection interleaves with attention on the
second half. b_attn is folded in on-device via K=1 bias matmuls; b_proj is
added on the host during the unshard reduction.
"""

import sys
import types

import numpy as np

B, T, C, H, D = 4, 2048, 1024, 16, 64
HG = 8            # heads per core
CG = HG * D       # 512 channels per group
NCORES = 8
TB = T // 128     # 16 t-blocks
QCH = T // 512    # 4 t-quarters


def _register_ntff_hook():
    """Register the axon NTFF profile hook if the image's antenv lacks it."""
    try:
        import antenv
        if getattr(antenv, "axon_hooks", None) is not None:
            return
        from trn_agent_boot.trn_boot import _ntff_profile_via_ctypes
        hook = _ntff_profile_via_ctypes("/opt/axon/libaxon_pjrt.so")
        mod = types.ModuleType("antenv.axon_hooks")
        mod._hook = hook
        mod.get_axon_ntff_profile_hook = lambda: mod._hook
        mod.set_axon_ntff_profile_hook = lambda h: setattr(mod, "_hook", h)
        sys.modules["antenv.axon_hooks"] = mod
        antenv.axon_hooks = mod
    except Exception:
        pass


_NC_CACHE = {}


def _build():
    import concourse.bacc as bacc
    import concourse.mybir as mybir
    import concourse.tile as tile
    from concourse.masks import make_upper_triangular
    from contextlib import ExitStack

    F32 = mybir.dt.float32
    F32R = mybir.dt.float32r
    BF16 = mybir.dt.bfloat16
    MUL = mybir.AluOpType.mult
    EXP = mybir.ActivationFunctionType.Exp

    nc = bacc.Bacc(None, target_bir_lowering=False, debug=False)
    xT_d = nc.dram_tensor("xT", [C, T], BF16, kind="ExternalInput")
    wqk_d = nc.dram_tensor("wqk", [C, 2 * CG], BF16, kind="ExternalInput")
    wv_d = nc.dram_tensor("wv", [C, CG], BF16, kind="ExternalInput")
    wp_d = nc.dram_tensor("wp", [CG, C], BF16, kind="ExternalInput")
    bqk_d = nc.dram_tensor("bqk", [1, 2 * CG], BF16, kind="ExternalInput")
    bv_d = nc.dram_tensor("bv", [1, CG], BF16, kind="ExternalInput")
    out_d = nc.dram_tensor("out", [T, C], F32, kind="ExternalOutput")

    CT = C // 128  # 8 c-tiles of the contraction dim

    with tile.TileContext(nc) as tc, ExitStack() as ctx:
        pers = ctx.enter_context(tc.tile_pool(name="pers", bufs=1))

        # Per-head qT/kT tiles in [d, t] layout. Head h's 64 d-rows live at
        # partitions (h%2)*64..(h%2)*64+64 (matching the projection PSUM
        # layout); the other 64 partitions are zeroed so QK matmuls contract
        # over a full K=128 (keeps the PE HAM clock warm; zeros add nothing).
        qTt = [pers.tile([128, T], BF16, name=f"qTt{h}") for h in range(HG)]
        kTt = [pers.tile([128, T], BF16, name=f"kTt{h}") for h in range(HG)]
        for h in range(HG):
            zs = slice(64, 128) if h % 2 == 0 else slice(0, 64)
            nc.gpsimd.memset(qTt[h][zs, :], 0.0)
            nc.gpsimd.memset(kTt[h][zs, :], 0.0)
        # v_aug[p, j, h, 0:64] = v[t=j*128+p, h*64+d]; [..., 64] = 1.0
        v_aug = pers.tile([128, TB, HG, 65], BF16, name="v_aug")
        utri = pers.tile([128, 128], BF16, name="utri")
        ones_col = pers.tile([1, 64], F32R, name="ones_col")
        ones_q = pers.tile([1, 512], BF16, name="ones_q")
        bqk_sb = pers.tile([1, 2 * CG], BF16, name="bqk_sb")
        bv_sb = pers.tile([1, CG], BF16, name="bv_sb")

        # bf16/f32r constants staged via f32 memset + rounding copies.
        stage = pers.tile([128, 512], F32, name="stage")
        make_upper_triangular(nc, utri[:, :], val=1.0, diag=True)
        nc.vector.memset(stage[:], 1.0)
        nc.vector.tensor_copy(ones_col[:], stage[0:1, 0:64])
        nc.vector.tensor_copy(ones_q[:], stage[0:1, :])
        nc.vector.tensor_copy(
            v_aug[:, :, :, 64:65],
            stage[:, 0:128].rearrange("p (j h) -> p j h", j=TB))
        nc.sync.dma_start(bqk_sb[:], bqk_d.ap()[:])
        nc.sync.dma_start(bv_sb[:], bv_d.ap()[:])

        wp_pool = ctx.enter_context(tc.tile_pool(name="wp_pool", bufs=1))
        wp_sb = [wp_pool.tile([128, C], BF16, name=f"wp{i}") for i in range(4)]
        yT_pool = ctx.enter_context(tc.tile_pool(name="yT_pool", bufs=1))
        yT = [yT_pool.tile([128, T], BF16, name=f"yT{i}") for i in range(4)]

        att_pool = ctx.enter_context(tc.tile_pool(name="att_pool", bufs=10))
        nrm_pool = ctx.enter_context(tc.tile_pool(name="nrm_pool", bufs=4))
        out_pool = ctx.enter_context(tc.tile_pool(name="out_pool", bufs=2))
        ps_s_pool = ctx.enter_context(
            tc.tile_pool(name="ps_s_pool", bufs=2, space="PSUM"))
        ps_y_pool = ctx.enter_context(
            tc.tile_pool(name="ps_y_pool", bufs=2, space="PSUM"))

        # Phase-1 working pools (manually released once quarters 2-3 finish).
        wqk_pool = tc.alloc_tile_pool(name="wqk_pool", bufs=1)
        wv_pool = tc.alloc_tile_pool(name="wv_pool", bufs=1)
        xq_pool = tc.alloc_tile_pool(name="xq_pool", bufs=2)
        wqk_sb = [wqk_pool.tile([128, 2 * CG], BF16, name=f"wqk{c}")
                  for c in range(CT)]
        wv_sb = [wv_pool.tile([128, CG], BF16, name=f"wv{c}") for c in range(CT)]

        # Spread bulk input DMAs over several engines' HWDGE queues so the
        # startup load isn't serialized on one queue.
        dma_engines = [nc.sync, nc.sync, nc.sync, nc.sync]
        for c in range(CT):
            dma_engines[c % 4].dma_start(
                wv_sb[c][:], wv_d.ap()[c * 128:(c + 1) * 128, :])

        xq_by_q = {}

        def p1_dma(q):
            xq = []
            for c in range(CT):
                xt = xq_pool.tile([128, 512], BF16, name=f"xq{c}", tag=f"xq{c}")
                dma_engines[c % 4].dma_start(
                    xt[:], xT_d.ap()[c * 128:(c + 1) * 128, q * 512:(q + 1) * 512])
                xq.append(xt)
            xq_by_q[q] = xq

        def p1_v_unit(q, tb):
            """V projection for t-block tb of quarter q."""
            xq = xq_by_q[q]
            pv = ps_s_pool.tile([128, CG], F32, name="pv", tag="ps_s")
            for c in range(CT):
                nc.tensor.matmul(
                    pv[:], xq[c][:, tb * 128:(tb + 1) * 128], wv_sb[c][:],
                    start=(c == 0), stop=False)
            nc.tensor.matmul(
                pv[:], ones_q[:, tb * 128:(tb + 1) * 128], bv_sb[:],
                start=False, stop=True)
            j = q * 4 + tb
            nc.vector.tensor_copy(
                v_aug[:, j, :, 0:64], pv[:].rearrange("p (h d) -> p h d", h=HG))

        def p1_qk_unit(q, m):
            """Q/K projection M-block m (heads 2(m%4), 2(m%4)+1) of quarter q."""
            xq = xq_by_q[q]
            pqk = ps_s_pool.tile([128, 512], F32, name="pqk", tag="ps_s")
            for c in range(CT):
                nc.tensor.matmul(
                    pqk[:], wqk_sb[c][:, m * 128:(m + 1) * 128], xq[c][:],
                    start=(c == 0), stop=False)
            nc.tensor.matmul(
                pqk[:], bqk_sb[:, m * 128:(m + 1) * 128], ones_q[:],
                start=False, stop=True)
            dst = qTt if m < 4 else kTt
            h0 = 2 * (m % 4)
            sl = slice(q * 512, (q + 1) * 512)
            nc.vector.tensor_copy(dst[h0][0:64, sl], pqk[0:64, :])
            nc.vector.tensor_copy(dst[h0 + 1][64:128, sl], pqk[64:128, :])

        def p1_units(q):
            for tb in range(4):
                yield lambda tb=tb: p1_v_unit(q, tb)
            for m in range(8):
                yield lambda m=m: p1_qk_unit(q, m)

        def normalize(ps_y, h, cch):
            # yT[d, q] /= sums[q] (sums live in the ones-row 64 of ps_y).
            sums_sb = nrm_pool.tile([1, 512], F32R, tag="sums")
            nc.vector.tensor_copy(sums_sb[:], ps_y[64:65, :])
            ps_b = ps_y_pool.tile([64, 512], F32, name="ps_b", tag="ps_b", bufs=1)
            nc.tensor.matmul(ps_b[:], ones_col[:], sums_sb[:],
                             start=True, stop=True)
            inv_sb = nrm_pool.tile([64, 512], F32, tag="inv")
            nc.vector.reciprocal_approx_fast(inv_sb[:], ps_b[:])
            ct, sl = h // 2, slice(cch * 512, (cch + 1) * 512)
            if h % 2 == 0:
                nc.vector.tensor_tensor(
                    out=yT[ct][0:64, sl], in0=ps_y[0:64, :],
                    in1=inv_sb[:], op=MUL)
            else:
                ystg = nrm_pool.tile([64, 512], BF16, tag="ystg")
                nc.vector.tensor_tensor(
                    out=ystg[:], in0=ps_y[0:64, :], in1=inv_sb[:], op=MUL)
                nc.sync.dma_start(yT[ct][64:128, sl], ystg[:])

        def attn_steps(h, c2, j, ps_y0, ps_y1):
            """QK -> exp -> mask -> AV for one (head, 1024-superchunk, j)."""
            jmax = 8 * c2 + 7
            q0 = c2 * 1024
            dead = (j - 8 * c2) * 128 if j >= 8 * c2 else 0
            ps_s = ps_s_pool.tile([128, 1024], F32, name="ps_s", tag="ps_s")
            if dead < 512:
                nc.tensor.matmul(
                    ps_s[:, dead:512],
                    kTt[h][:, j * 128:(j + 1) * 128],
                    qTt[h][:, q0 + dead:q0 + 512],
                    start=True, stop=True)
            lo_s = max(512, dead)
            nc.tensor.matmul(
                ps_s[:, lo_s:1024],
                kTt[h][:, j * 128:(j + 1) * 128],
                qTt[h][:, q0 + lo_s:q0 + 1024],
                start=True, stop=True)
            att = att_pool.tile([128, 1024], BF16, tag="att")
            nc.scalar.activation(
                att[:, dead:1024], ps_s[:, dead:1024], EXP, scale=0.125)
            if j >= 8 * c2:
                nc.vector.tensor_tensor(
                    out=att[:, dead:dead + 128], in0=att[:, dead:dead + 128],
                    in1=utri[:, :], op=MUL)
            if j <= 8 * c2 + 3:
                nc.tensor.matmul(
                    ps_y0[:, dead:512], v_aug[:, j, h, :], att[:, dead:512],
                    start=(j == 0), stop=(j == 8 * c2 + 3))
            lo1 = max(512, dead)
            nc.tensor.matmul(
                ps_y1[:, lo1 - 512:512], v_aug[:, j, h, :], att[:, lo1:1024],
                start=(j == 0), stop=(j == jmax))

        def attn_head(h, c2):
            ps_y0 = ps_y_pool.tile([65, 512], F32, name="ps_y0", tag="ps_y")
            ps_y1 = ps_y_pool.tile([65, 512], F32, name="ps_y1", tag="ps_y")
            for j in range(8 * c2 + 8):
                attn_steps(h, c2, j, ps_y0, ps_y1)
            normalize(ps_y0, h, 2 * c2)
            normalize(ps_y1, h, 2 * c2 + 1)

        def proj_unit(tb, ptag="pp"):
            o_sb = out_pool.tile([128, C], F32, tag="o_sb")
            for ch in range(2):
                pp = ps_y_pool.tile([128, 512], F32, name="pp", tag=ptag, bufs=1)
                for ct in range(4):
                    nc.tensor.matmul(
                        pp[:],
                        yT[ct][:, tb * 128:(tb + 1) * 128],
                        wp_sb[ct][:, ch * 512:(ch + 1) * 512],
                        start=(ct == 0), stop=(ct == 3))
                nc.vector.tensor_copy(o_sb[:, ch * 512:(ch + 1) * 512], pp[:])
            nc.sync.dma_start(out_d.ap()[tb * 128:(tb + 1) * 128, :], o_sb[:])

        # ---------------- Orchestration ----------------
        # Quarters 0-1 straight through.
        p1_dma(0)
        for c in range(CT):
            dma_engines[c % 4].dma_start(
                wqk_sb[c][:], wqk_d.ap()[c * 128:(c + 1) * 128, :])
        p1_dma(1)
        for u in p1_units(0):
            u()
        for u in p1_units(1):
            u()
        for i in range(4):
            nc.sync.dma_start(wp_sb[i][:], wp_d.ap()[i * 128:(i + 1) * 128, :])

        # Attention on q < 1024 interleaved with projection quarters 2-3.
        p1_dma(2)
        rest = list(p1_units(2))
        emitted_dma3 = False
        for h in range(HG):
            attn_head(h, 0)
            if not emitted_dma3:
                p1_dma(3)
                rest += list(p1_units(3))
                emitted_dma3 = True
            take, rest = rest[:3], rest[3:]
            for u in take:
                u()
        for u in rest:
            u()
        xq_pool.release()
        wv_pool.release()
        wqk_pool.release()

        # Attention on q >= 1024 interleaved with the ready half of the
        # output projection (t < 1024 only needs yT chunks 0-1).
        for h in range(HG):
            attn_head(h, 1)
            proj_unit(h, "pp")
        for tb in range(8, 16):
            proj_unit(tb, "pp" if tb % 2 else "ps_b")

    nc.compile()
    return nc


def _get_nc():
    if "nc" not in _NC_CACHE:
        _register_ntff_hook()
        _NC_CACHE["nc"] = _build()
    return _NC_CACHE["nc"]


def kernel(x, w_attn, b_attn, w_proj, b_proj, _run_kwargs=None):
    import ml_dtypes
    from concourse.bass_utils import run_bass_kernel_spmd

    bf16 = ml_dtypes.bfloat16
    x = np.asarray(x, dtype=np.float32)
    w_attn = np.asarray(w_attn, dtype=np.float32)
    b_attn = np.asarray(b_attn, dtype=np.float32)
    w_proj = np.asarray(w_proj, dtype=np.float32)
    b_proj = np.asarray(b_proj, dtype=np.float32)

    nc = _get_nc()
    in_maps = []
    for core in range(NCORES):
        b, g = divmod(core, 2)
        cols = slice(g * CG, (g + 1) * CG)
        in_maps.append({
            "xT": np.ascontiguousarray(x[b].T).astype(bf16),
            "wqk": np.concatenate(
                [w_attn[:, cols], w_attn[:, C + g * CG: C + (g + 1) * CG]],
                axis=1).astype(bf16),
            "wv": np.ascontiguousarray(
                w_attn[:, 2 * C + g * CG: 2 * C + (g + 1) * CG]).astype(bf16),
            "wp": np.ascontiguousarray(w_proj[g * CG:(g + 1) * CG, :]).astype(bf16),
            "bqk": np.concatenate(
                [b_attn[cols], b_attn[C + g * CG: C + (g + 1) * CG]]
            ).reshape(1, -1).astype(bf16),
            "bv": np.ascontiguousarray(
                b_attn[2 * C + g * CG: 2 * C + (g + 1) * CG]).reshape(1, -1).astype(bf16),
        })

    res = run_bass_kernel_spmd(nc, in_maps, core_ids=list(range(NCORES)),
                               **(_run_kwargs or {}))
    out = np.empty((B, T, C), dtype=np.float32)
    for b in range(B):
        out[b] = res.results[2 * b]["out"] + res.results[2 * b + 1]["out"] + b_proj
    if _run_kwargs:
        kernel.last_results = res
    return out



# revision 11
# speedup vs baseline: 1.2366x; 1.2366x over previous
"""Causal self-attention (B=4, T=2048, C=1024, H=16, D=64) on 8 TRN2 NeuronCores.

Sharding: 8 cores = 4 batches x 2 head-groups (8 heads each). Each core:
  - QKV projection for its (batch, head-group) column slice of w_attn,
    producing qT/kT in [d, t] layout and v in [t, d].
  - Causal attention with softmax denominators from 64 replicated
    ones-columns appended to V (no cross-partition reduction needed).
  - Row-sharded output projection -> per-core partial [T, C].
Host sums the two partials per batch and adds b_proj.

Head-pair layout: heads (2m, 2m+1) share one [128, T] qT/kT tile
(partitions 0-63 / 64-127), so the two K=64 QK matmuls of a pair run
concurrently as row-tiles of the PE array (tile_position (0,0)/(64,0)),
writing the two 512-col halves of one [128, 1024] PSUM tile.  A single
exp activation then covers both heads, halving ACT instruction count.
Attention is processed in 512-wide q-chunks; chunk c only needs
projection quarter c of Q (and quarters <= c of K/V), so attention
starts right after quarter 0 and the projection quarters + the output
projection interleave with it to keep the PE busy while ACT runs exp.

Matmul operands are bf16 (1 cycle/row on the PE) with all accumulation
in fp32 PSUM.  Input DMAs are batched into ~10 large descriptors spread
over the SP and ACT HWDGE rings.
"""

import sys
import types

import numpy as np

B, T, C, H, D = 4, 2048, 1024, 16, 64
HG = 8            # heads per core
CG = HG * D       # 512 channels per group
NP = HG // 2      # 4 head-pairs per core
NCORES = 8
TB = T // 128     # 16 t-blocks (also k-blocks)
CT = C // 128     # 8 c-chunks of the contraction dim
NCH = 4           # 512-wide q-chunks


def _register_ntff_hook():
    """Register the axon NTFF profile hook if the image's antenv lacks it."""
    try:
        import antenv
        if getattr(antenv, "axon_hooks", None) is not None:
            return
        from trn_agent_boot.trn_boot import _ntff_profile_via_ctypes
        hook = _ntff_profile_via_ctypes("/opt/axon/libaxon_pjrt.so")
        mod = types.ModuleType("antenv.axon_hooks")
        mod._hook = hook
        mod.get_axon_ntff_profile_hook = lambda: mod._hook
        mod.set_axon_ntff_profile_hook = lambda h: setattr(mod, "_hook", h)
        sys.modules["antenv.axon_hooks"] = mod
        antenv.axon_hooks = mod
    except Exception:
        pass


_NC_CACHE = {}


def _build():
    import concourse.bacc as bacc
    import concourse.mybir as mybir
    import concourse.tile as tile
    from concourse.masks import make_upper_triangular
    from contextlib import ExitStack

    F32 = mybir.dt.float32
    BF16 = mybir.dt.bfloat16
    MUL = mybir.AluOpType.mult
    EXP = mybir.ActivationFunctionType.Exp

    nc = bacc.Bacc(None, target_bir_lowering=False, debug=False)
    xT_d = nc.dram_tensor("xT", [C, T], BF16, kind="ExternalInput")
    wqk_d = nc.dram_tensor("wqk", [C, 2 * CG], BF16, kind="ExternalInput")
    wv_d = nc.dram_tensor("wv", [C, CG], BF16, kind="ExternalInput")
    wp_d = nc.dram_tensor("wp", [CG, C], BF16, kind="ExternalInput")
    bqkT_d = nc.dram_tensor("bqkT", [128, 8], F32, kind="ExternalInput")
    bv_d = nc.dram_tensor("bv", [1, CG], BF16, kind="ExternalInput")
    out_d = nc.dram_tensor("out", [T, C], F32, kind="ExternalOutput")
    import os
    dbg = {}
    if os.environ.get("KDBG"):
        dbg["qTp0"] = nc.dram_tensor("d_qTp0", [128, T], BF16, kind="ExternalOutput")
        dbg["kTp0"] = nc.dram_tensor("d_kTp0", [128, T], BF16, kind="ExternalOutput")
        dbg["v_aug"] = nc.dram_tensor("d_vaug", [128, TB * HG * 128], BF16, kind="ExternalOutput")
        for m in range(NP):
            dbg[f"yT{m}"] = nc.dram_tensor(f"d_yT{m}", [128, T], BF16, kind="ExternalOutput")

    with tile.TileContext(nc) as tc, ExitStack() as ctx:
        pers = ctx.enter_context(tc.tile_pool(name="pers", bufs=1))

        # Head-pair qT/kT tiles in [d, t] layout: head 2m at partitions
        # 0-63, head 2m+1 at partitions 64-127.
        qTp = [pers.tile([128, T], BF16, name=f"qTp{m}") for m in range(NP)]
        kTp = [pers.tile([128, T], BF16, name=f"kTp{m}") for m in range(NP)]
        # v_aug[p, j, h, 0:64] = 1.0; [..., 64:128] = v[t=j*128+p, h*64+d]
        # (64 replicated ones-columns make the AV matmul emit the softmax
        # denominator on partitions 0-63 — base-0 so reciprocal_approx_fast
        # can read it directly; the custom-DVE op breaks on partition-offset
        # APs).
        v_aug = pers.tile([128, TB, HG, 128], BF16, name="v_aug")
        utri = pers.tile([128, 128], BF16, name="utri")
        ones_q = pers.tile([1, 128], BF16, name="ones_q")
        bqkT_sb = pers.tile([128, 8], F32, name="bqkT_sb")
        bv_sb = pers.tile([1, CG], BF16, name="bv_sb")

        wqk_sb = pers.tile([128, CT, 2 * CG], BF16, name="wqk_sb")
        wv_sb = pers.tile([128, CT, CG], BF16, name="wv_sb")
        wp_sb = pers.tile([128, NP, C], BF16, name="wp_sb")
        yT = [pers.tile([128, T], BF16, name=f"yT{m}") for m in range(NP)]

        # Constants (emitted first so their tile-ranges are settled before
        # the v-copies / masks that share tiles with them).
        for j in range(TB):
            nc.vector.memset(v_aug[:, j, :, 0:64], 1.0)
        make_upper_triangular(nc, utri[:, :], val=1.0, diag=True)
        nc.vector.memset(ones_q[:], 1.0)

        # ---- input DMAs: few large descriptors, two HWDGE rings ----
        xq_pool = ctx.enter_context(tc.tile_pool(name="xq_pool", bufs=2))
        xq_by_q = {}

        def p1_dma(q):
            xq = xq_pool.tile([128, CT, 512], BF16, name="xq", tag="xq")
            nc.sync.dma_start(
                xq[:],
                xT_d.ap()[:, q * 512:(q + 1) * 512].rearrange(
                    "(c p) t -> p c t", p=128))
            xq_by_q[q] = xq

        p1_dma(0)
        nc.scalar.dma_start(
            wv_sb[:], wv_d.ap().rearrange("(c p) n -> p c n", p=128))
        nc.scalar.dma_start(bqkT_sb[:], bqkT_d.ap()[:])
        nc.scalar.dma_start(bv_sb[:], bv_d.ap()[:])
        nc.scalar.dma_start(
            wqk_sb[:, :, 0:CG],
            wqk_d.ap()[:, 0:CG].rearrange("(c p) n -> p c n", p=128))
        p1_dma(1)
        nc.scalar.dma_start(
            wqk_sb[:, :, CG:2 * CG],
            wqk_d.ap()[:, CG:2 * CG].rearrange("(c p) n -> p c n", p=128))
        nc.scalar.dma_start(
            wp_sb[:], wp_d.ap().rearrange("(ct p) n -> p ct n", p=128))
        p1_dma(2)
        p1_dma(3)

        # ---- PSUM pools ----
        sc_pool = ctx.enter_context(
            tc.tile_pool(name="sc_pool", bufs=2, space="PSUM"))   # 4 banks
        ps_y_pool = ctx.enter_context(
            tc.tile_pool(name="ps_y_pool", bufs=1, space="PSUM"))  # 2 banks
        pp_pool = ctx.enter_context(
            tc.tile_pool(name="pp_pool", bufs=2, space="PSUM"))   # 2 banks

        att_pool = ctx.enter_context(tc.tile_pool(name="att_pool", bufs=3))
        nrm_pool = ctx.enter_context(tc.tile_pool(name="nrm_pool", bufs=2))
        out_pool = ctx.enter_context(tc.tile_pool(name="out_pool", bufs=2))

        # ---- projection units ----
        def p1_v_unit(q, tb):
            """V projection for t-block tb of quarter q."""
            xq = xq_by_q[q]
            pv = pp_pool.tile([128, CG], F32, name="pv", tag="pp")
            for c in range(CT):
                nc.tensor.matmul(
                    pv[:], xq[:, c, tb * 128:(tb + 1) * 128], wv_sb[:, c, :],
                    start=(c == 0), stop=False)
            nc.tensor.matmul(
                pv[:], ones_q[:], bv_sb[:], start=False, stop=True)
            j = q * 4 + tb
            nc.vector.tensor_copy(
                v_aug[:, j, :, 64:128], pv[:].rearrange("p (h d) -> p h d", h=HG))

        def p1_qk_unit(q, m):
            """Q (m<4) or K (m>=4) projection block m of quarter q."""
            xq = xq_by_q[q]
            pqk = pp_pool.tile([128, 512], F32, name="pqk", tag="pp")
            for c in range(CT):
                nc.tensor.matmul(
                    pqk[:], wqk_sb[:, c, m * 128:(m + 1) * 128], xq[:, c, :],
                    start=(c == 0), stop=(c == CT - 1))
            dst = qTp[m] if m < NP else kTp[m - NP]
            sl = slice(q * 512, (q + 1) * 512)
            nc.vector.tensor_scalar_add(dst[:, sl], pqk[:], bqkT_sb[:, m:m + 1])

        def p1_units(q):
            for tb in range(4):
                yield lambda tb=tb: p1_v_unit(q, tb)
            for m in range(8):
                yield lambda m=m: p1_qk_unit(q, m)

        # ---- attention ----
        def attn_step(p, cch, j, ps_y2, first, last):
            """QK pair -> exp -> mask -> AV pair for one (pair, chunk, j)."""
            dead = max(0, j - 4 * cch) * 128
            q0 = cch * 512
            ps = sc_pool.tile([128, 1024], F32, name="ps", tag="sc")
            nc.tensor.matmul(
                ps[:, dead:512],
                kTp[p][0:64, j * 128:(j + 1) * 128],
                qTp[p][0:64, q0 + dead:q0 + 512],
                start=True, stop=True)
            nc.tensor.matmul(
                ps[:, 512 + dead:1024],
                kTp[p][64:128, j * 128:(j + 1) * 128],
                qTp[p][64:128, q0 + dead:q0 + 512],
                start=True, stop=True)
            att = att_pool.tile([128, 1024], BF16, tag="att")
            nc.scalar.activation(
                att[:, dead:1024], ps[:, dead:1024], EXP, scale=0.125)
            if j >= 4 * cch:
                nc.vector.tensor_tensor(
                    out=att[:, dead:dead + 128], in0=att[:, dead:dead + 128],
                    in1=utri[:, :], op=MUL)
                nc.vector.tensor_tensor(
                    out=att[:, 512 + dead:640 + dead],
                    in0=att[:, 512 + dead:640 + dead],
                    in1=utri[:, :], op=MUL)
            nc.tensor.matmul(
                ps_y2[0][:, dead:512], v_aug[:, j, 2 * p, :],
                att[:, dead:512], start=first, stop=last)
            nc.tensor.matmul(
                ps_y2[1][:, dead:512], v_aug[:, j, 2 * p + 1, :],
                att[:, 512 + dead:1024], start=first, stop=last)

        def normalize(p, cch, ps_y2):
            sl = slice(cch * 512, (cch + 1) * 512)
            for hh in range(2):
                inv = nrm_pool.tile([64, 512], F32, tag="inv")
                nc.vector.reciprocal_approx_fast(inv[:], ps_y2[hh][0:64, :])
                rows = slice(0, 64) if hh == 0 else slice(64, 128)
                nc.vector.tensor_tensor(
                    out=yT[p][rows, sl], in0=ps_y2[hh][64:128, :],
                    in1=inv[:], op=MUL)

        def attn_chunk_pair(p, cch, interleave):
            """All j-steps of (pair, chunk); calls interleave() between."""
            ps_y2 = [ps_y_pool.tile([128, 512], F32, name=f"ps_y{hh}",
                                    tag=f"ps_y{hh}") for hh in range(2)]
            jmax = 4 * cch + 3
            for j in range(jmax + 1):
                attn_step(p, cch, j, ps_y2, first=(j == 0), last=(j == jmax))
                interleave()
            normalize(p, cch, ps_y2)

        # ---- output projection ----
        def proj_unit(tb):
            o_sb = out_pool.tile([128, C], F32, tag="o_sb")
            for ch in range(2):
                pp = pp_pool.tile([128, 512], F32, name="pp", tag="pp")
                for ct in range(NP):
                    nc.tensor.matmul(
                        pp[:],
                        yT[ct][:, tb * 128:(tb + 1) * 128],
                        wp_sb[:, ct, ch * 512:(ch + 1) * 512],
                        start=(ct == 0), stop=(ct == NP - 1))
                nc.vector.tensor_copy(o_sb[:, ch * 512:(ch + 1) * 512], pp[:])
            nc.sync.dma_start(out_d.ap()[tb * 128:(tb + 1) * 128, :], o_sb[:])

        # ---------------- Orchestration ----------------
        # Quarter 0 straight through; attention chunk c needs Q quarter c
        # and K/V quarters <= c, so chunk c overlaps projection quarter
        # c+1 and the output projection of earlier chunks.
        for u in p1_units(0):
            u()

        fill_by_chunk = {
            0: list(p1_units(1)),
            1: list(p1_units(2)) + [lambda tb=tb: proj_unit(tb)
                                    for tb in range(0, 4)],
            2: list(p1_units(3)),
            3: [lambda tb=tb: proj_unit(tb) for tb in range(4, 12)],
        }

        for cch in range(NCH):
            fill = fill_by_chunk[cch]
            nsteps = NP * (4 * cch + 4)
            quota = [0] * nsteps
            for i in range(len(fill)):
                quota[(i * nsteps) // len(fill)] += 1
            it = iter(range(nsteps))
            step_i = [0]

            def interleave():
                i = step_i[0]
                step_i[0] += 1
                for _ in range(quota[i] if i < nsteps else 0):
                    fill.pop(0)()

            for p in range(NP):
                attn_chunk_pair(p, cch, interleave)
            for u in fill:
                u()

        for tb in range(12, 16):
            proj_unit(tb)

        if dbg:
            nc.sync.dma_start(dbg["qTp0"].ap()[:], qTp[0][:])
            nc.sync.dma_start(dbg["kTp0"].ap()[:], kTp[0][:])
            nc.sync.dma_start(
                dbg["v_aug"].ap()[:],
                v_aug[:].rearrange("p j h d -> p (j h d)"))
            for m in range(NP):
                nc.sync.dma_start(dbg[f"yT{m}"].ap()[:], yT[m][:])

    nc.compile()
    return nc


def _get_nc():
    if "nc" not in _NC_CACHE:
        _register_ntff_hook()
        _NC_CACHE["nc"] = _build()
    return _NC_CACHE["nc"]


def kernel(x, w_attn, b_attn, w_proj, b_proj, _run_kwargs=None):
    import ml_dtypes
    from concourse.bass_utils import run_bass_kernel_spmd

    bf16 = ml_dtypes.bfloat16
    x = np.asarray(x, dtype=np.float32)
    w_attn = np.asarray(w_attn, dtype=np.float32)
    b_attn = np.asarray(b_attn, dtype=np.float32)
    w_proj = np.asarray(w_proj, dtype=np.float32)
    b_proj = np.asarray(b_proj, dtype=np.float32)

    nc = _get_nc()
    in_maps = []
    for core in range(NCORES):
        b, g = divmod(core, 2)
        cols = slice(g * CG, (g + 1) * CG)
        bqk = np.concatenate(
            [b_attn[cols], b_attn[C + g * CG: C + (g + 1) * CG]])
        in_maps.append({
            "xT": np.ascontiguousarray(x[b].T).astype(bf16),
            "wqk": np.concatenate(
                [w_attn[:, cols], w_attn[:, C + g * CG: C + (g + 1) * CG]],
                axis=1).astype(bf16),
            "wv": np.ascontiguousarray(
                w_attn[:, 2 * C + g * CG: 2 * C + (g + 1) * CG]).astype(bf16),
            "wp": np.ascontiguousarray(w_proj[g * CG:(g + 1) * CG, :]).astype(bf16),
            "bqkT": np.ascontiguousarray(
                bqk.reshape(8, 128).T).astype(np.float32),
            "bv": np.ascontiguousarray(
                b_attn[2 * C + g * CG: 2 * C + (g + 1) * CG]).reshape(1, -1).astype(bf16),
        })

    res = run_bass_kernel_spmd(nc, in_maps, core_ids=list(range(NCORES)),
                               **(_run_kwargs or {}))
    out = np.empty((B, T, C), dtype=np.float32)
    for b in range(B):
        out[b] = res.results[2 * b]["out"] + res.results[2 * b + 1]["out"] + b_proj
    if _run_kwargs:
        kernel.last_results = res
    return out


# revision 18
# speedup vs baseline: 1.2634x; 1.0217x over previous
"""Causal self-attention (B=4, T=2048, C=1024, H=16, D=64) on 8 TRN2 NeuronCores.

Sharding: 8 cores = 4 batches x 2 head-groups (8 heads each). Each core:
  - QKV projection for its (batch, head-group) column slice of w_attn,
    producing qT/kT in [d, t] layout and v in [t, d].
  - Causal attention with softmax denominators from 64 replicated
    ones-columns appended to V (no cross-partition reduction needed).
  - Row-sharded output projection -> per-core partial [T, C].
Host sums the two partials per batch and adds b_proj.

Head-pair layout: heads (2m, 2m+1) share one [128, T] qT/kT tile
(partitions 0-63 / 64-127), so the two K=64 QK matmuls of a pair run
concurrently as row-tiles of the PE array (tile_position (0,0)/(64,0)),
writing the two 512-col halves of one [128, 1024] PSUM tile.  A single
exp activation then covers both heads, halving ACT instruction count.
Attention is processed in 512-wide q-chunks; chunk c only needs
projection quarter c of Q (and quarters <= c of K/V), so attention
starts right after quarter 0 and the projection quarters + the output
projection interleave with it to keep the PE busy while ACT runs exp.

Matmul operands are bf16 (1 cycle/row on the PE) with all accumulation
in fp32 PSUM.  Input DMAs are batched into ~10 large descriptors spread
over the SP and ACT HWDGE rings.
"""

import sys
import types

import numpy as np

B, T, C, H, D = 4, 2048, 1024, 16, 64
HG = 8            # heads per core
CG = HG * D       # 512 channels per group
NP = HG // 2      # 4 head-pairs per core
NCORES = 8
TB = T // 128     # 16 t-blocks (also k-blocks)
CT = C // 128     # 8 c-chunks of the contraction dim
NCH = 4           # 512-wide q-chunks


def _register_ntff_hook():
    """Register the axon NTFF profile hook if the image's antenv lacks it."""
    try:
        import antenv
        if getattr(antenv, "axon_hooks", None) is not None:
            return
        from trn_agent_boot.trn_boot import _ntff_profile_via_ctypes
        hook = _ntff_profile_via_ctypes("/opt/axon/libaxon_pjrt.so")
        mod = types.ModuleType("antenv.axon_hooks")
        mod._hook = hook
        mod.get_axon_ntff_profile_hook = lambda: mod._hook
        mod.set_axon_ntff_profile_hook = lambda h: setattr(mod, "_hook", h)
        sys.modules["antenv.axon_hooks"] = mod
        antenv.axon_hooks = mod
    except Exception:
        pass


_NC_CACHE = {}


def _build():
    import concourse.bacc as bacc
    import concourse.mybir as mybir
    import concourse.tile as tile
    from concourse.masks import make_upper_triangular
    from contextlib import ExitStack

    F32 = mybir.dt.float32
    BF16 = mybir.dt.bfloat16
    MUL = mybir.AluOpType.mult
    EXP = mybir.ActivationFunctionType.Exp

    nc = bacc.Bacc(None, target_bir_lowering=False, debug=False)
    # All inputs host-pre-arranged to [128, ...] partition-major layouts so
    # every DMA is a contiguous block (tiny descriptor count, line rate).
    # xq[p, q, c, t] = x[t=q*512+t, c*128+p]
    xq_d = nc.dram_tensor("xq", [128, NCH * CT * 512], BF16, kind="ExternalInput")
    # wqk[p, s, c, m, n] = w_attn[c*128+p, s*C + (4*s+m... see host prep
    wqk_d = nc.dram_tensor("wqk", [128, 2 * CT * CG], BF16, kind="ExternalInput")
    wv_d = nc.dram_tensor("wv", [128, CT * CG], BF16, kind="ExternalInput")
    wp_d = nc.dram_tensor("wp", [128, NP * C], BF16, kind="ExternalInput")
    bqkT_d = nc.dram_tensor("bqkT", [128, 8], F32, kind="ExternalInput")
    bv_d = nc.dram_tensor("bv", [1, CG], BF16, kind="ExternalInput")
    out_d = nc.dram_tensor("out", [T, C], BF16, kind="ExternalOutput")
    import os
    dbg = {}
    if os.environ.get("KDBG"):
        dbg["qTp0"] = nc.dram_tensor("d_qTp0", [128, T], BF16, kind="ExternalOutput")
        dbg["kTp0"] = nc.dram_tensor("d_kTp0", [128, T], BF16, kind="ExternalOutput")
        dbg["v_aug"] = nc.dram_tensor("d_vaug", [128, TB * HG * 128], BF16, kind="ExternalOutput")
        for m in range(NP):
            dbg[f"yT{m}"] = nc.dram_tensor(f"d_yT{m}", [128, T], BF16, kind="ExternalOutput")

    with tile.TileContext(nc) as tc, ExitStack() as ctx:
        pers = ctx.enter_context(tc.tile_pool(name="pers", bufs=1))

        # Head-pair qT/kT tiles in [d, t] layout: head 2m at partitions
        # 0-63, head 2m+1 at partitions 64-127.
        qTp = [pers.tile([128, T], BF16, name=f"qTp{m}") for m in range(NP)]
        kTp = [pers.tile([128, T], BF16, name=f"kTp{m}") for m in range(NP)]
        # v_aug[p, j, h, 0:64] = 1.0; [..., 64:128] = v[t=j*128+p, h*64+d]
        # (64 replicated ones-columns make the AV matmul emit the softmax
        # denominator on partitions 0-63 — base-0 so reciprocal_approx_fast
        # can read it directly; the custom-DVE op breaks on partition-offset
        # APs).
        v_aug = pers.tile([128, TB, HG, 128], BF16, name="v_aug")
        utri = pers.tile([128, 128], BF16, name="utri")
        ones_q = pers.tile([1, 128], BF16, name="ones_q")
        bqkT_sb = pers.tile([128, 8], F32, name="bqkT_sb")
        bv_sb = pers.tile([1, CG], BF16, name="bv_sb")

        wqk_sb = pers.tile([128, 2, CT, 512], BF16, name="wqk_sb")
        wv_sb = pers.tile([128, CT, CG], BF16, name="wv_sb")
        wp_sb = pers.tile([128, NP, C], BF16, name="wp_sb")
        yT = [pers.tile([128, T], BF16, name=f"yT{m}") for m in range(NP)]

        # Constants (emitted first so their tile-ranges are settled before
        # the v-copies / masks that share tiles with them).
        for j in range(TB):
            nc.vector.memset(v_aug[:, j, :, 0:64], 1.0)
        make_upper_triangular(nc, utri[:, :], val=1.0, diag=True)
        nc.vector.memset(ones_q[:], 1.0)

        # ---- input DMAs: contiguous blocks, two HWDGE rings ----
        xq_pool = ctx.enter_context(tc.tile_pool(name="xq_pool", bufs=2))
        xq_by_q = {}

        def p1_dma(q, split=1):
            xq = xq_pool.tile([128, CT, 512], BF16, name="xq", tag="xq")
            w = CT * 512
            for s in range(split):
                lo, hi = s * w // split, (s + 1) * w // split
                nc.sync.dma_start(
                    xq[:].rearrange("p c t -> p (c t)")[:, lo:hi],
                    xq_d.ap()[:, q * w + lo:q * w + hi])
            xq_by_q[q] = xq

        p1_dma(0, split=2)
        wvf = wv_sb[:].rearrange("p c n -> p (c n)")
        nc.scalar.dma_start(wvf[:, 0:2048], wv_d.ap()[:, 0:2048])
        nc.scalar.dma_start(wvf[:, 2048:4096], wv_d.ap()[:, 2048:4096])
        nc.scalar.dma_start(bv_sb[:], bv_d.ap()[:])
        # wqk_sb[p, s, c, n]: s=0 q-part (all c contiguous), s=1 k-part
        wqkf = wqk_sb[:].rearrange("p s c n -> p (s c n)")
        nc.scalar.dma_start(wqkf[:, 0:4096], wqk_d.ap()[:, 0:4096])
        p1_dma(1)
        nc.scalar.dma_start(wqkf[:, 4096:8192], wqk_d.ap()[:, 4096:8192])
        nc.scalar.dma_start(bqkT_sb[:], bqkT_d.ap()[:])
        nc.scalar.dma_start(
            wp_sb[:].rearrange("p ct n -> p (ct n)"), wp_d.ap()[:])
        p1_dma(2)
        p1_dma(3)

        # ---- PSUM pools ----
        sc_pool = ctx.enter_context(
            tc.tile_pool(name="sc_pool", bufs=2, space="PSUM"))   # 4 banks
        ps_y_pool = ctx.enter_context(
            tc.tile_pool(name="ps_y_pool", bufs=1, space="PSUM"))  # 2 banks
        pp_pool = ctx.enter_context(
            tc.tile_pool(name="pp_pool", bufs=2, space="PSUM"))   # 2 banks

        att_pool = ctx.enter_context(tc.tile_pool(name="att_pool", bufs=3))
        nrm_pool = ctx.enter_context(tc.tile_pool(name="nrm_pool", bufs=2))
        out_pool = ctx.enter_context(tc.tile_pool(name="out_pool", bufs=2))

        # ---- projection units ----
        def p1_v_unit(q, tb):
            """V projection for t-block tb of quarter q."""
            xq = xq_by_q[q]
            pv = pp_pool.tile([128, CG], F32, name="pv", tag="pp")
            for c in range(CT):
                nc.tensor.matmul(
                    pv[:], xq[:, c, tb * 128:(tb + 1) * 128], wv_sb[:, c, :],
                    start=(c == 0), stop=False)
            nc.tensor.matmul(
                pv[:], ones_q[:], bv_sb[:], start=False, stop=True)
            j = q * 4 + tb
            nc.vector.tensor_copy(
                v_aug[:, j, :, 64:128], pv[:].rearrange("p (h d) -> p h d", h=HG))

        def p1_qk_unit(q, m):
            """Q (m<4) or K (m>=4) projection block m of quarter q."""
            xq = xq_by_q[q]
            pqk = pp_pool.tile([128, 512], F32, name="pqk", tag="pp")
            s, mm = divmod(m, NP)
            for c in range(CT):
                nc.tensor.matmul(
                    pqk[:], wqk_sb[:, s, c, mm * 128:(mm + 1) * 128], xq[:, c, :],
                    start=(c == 0), stop=(c == CT - 1))
            dst = qTp[m] if m < NP else kTp[m - NP]
            sl = slice(q * 512, (q + 1) * 512)
            nc.vector.tensor_scalar_add(dst[:, sl], pqk[:], bqkT_sb[:, m:m + 1])

        def p1_units(q):
            for tb in range(4):
                yield lambda tb=tb: p1_v_unit(q, tb)
            for m in range(8):
                yield lambda m=m: p1_qk_unit(q, m)

        # ---- attention ----
        def attn_step(p, cch, j, ps_y2, first, last):
            """QK pair -> exp -> mask -> AV pair for one (pair, chunk, j)."""
            dead = max(0, j - 4 * cch) * 128
            q0 = cch * 512
            ps = sc_pool.tile([128, 1024], F32, name="ps", tag="sc")
            nc.tensor.matmul(
                ps[:, dead:512],
                kTp[p][0:64, j * 128:(j + 1) * 128],
                qTp[p][0:64, q0 + dead:q0 + 512],
                start=True, stop=True)
            nc.tensor.matmul(
                ps[:, 512 + dead:1024],
                kTp[p][64:128, j * 128:(j + 1) * 128],
                qTp[p][64:128, q0 + dead:q0 + 512],
                start=True, stop=True)
            att = att_pool.tile([128, 1024], BF16, tag="att")
            nc.scalar.activation(
                att[:, dead:1024], ps[:, dead:1024], EXP, scale=0.125)
            if j >= 4 * cch:
                nc.vector.tensor_tensor(
                    out=att[:, dead:dead + 128], in0=att[:, dead:dead + 128],
                    in1=utri[:, :], op=MUL)
                nc.vector.tensor_tensor(
                    out=att[:, 512 + dead:640 + dead],
                    in0=att[:, 512 + dead:640 + dead],
                    in1=utri[:, :], op=MUL)
            nc.tensor.matmul(
                ps_y2[0][:, dead:512], v_aug[:, j, 2 * p, :],
                att[:, dead:512], start=first, stop=last)
            nc.tensor.matmul(
                ps_y2[1][:, dead:512], v_aug[:, j, 2 * p + 1, :],
                att[:, 512 + dead:1024], start=first, stop=last)

        def normalize(p, cch, ps_y2):
            sl = slice(cch * 512, (cch + 1) * 512)
            for hh in range(2):
                inv = nrm_pool.tile([64, 512], F32, tag="inv")
                nc.vector.reciprocal_approx_fast(inv[:], ps_y2[hh][0:64, :])
                rows = slice(0, 64) if hh == 0 else slice(64, 128)
                nc.vector.tensor_tensor(
                    out=yT[p][rows, sl], in0=ps_y2[hh][64:128, :],
                    in1=inv[:], op=MUL)

        def attn_chunk_pair(p, cch, interleave):
            """All j-steps of (pair, chunk); calls interleave() between."""
            ps_y2 = [ps_y_pool.tile([128, 512], F32, name=f"ps_y{hh}",
                                    tag=f"ps_y{hh}") for hh in range(2)]
            jmax = 4 * cch + 3
            for j in range(jmax + 1):
                attn_step(p, cch, j, ps_y2, first=(j == 0), last=(j == jmax))
                interleave()
            normalize(p, cch, ps_y2)

        # ---- output projection ----
        def proj_unit(tb):
            o_sb = out_pool.tile([128, C], BF16, tag="o_sb")
            for ch in range(2):
                pp = pp_pool.tile([128, 512], F32, name="pp", tag="pp")
                for ct in range(NP):
                    nc.tensor.matmul(
                        pp[:],
                        yT[ct][:, tb * 128:(tb + 1) * 128],
                        wp_sb[:, ct, ch * 512:(ch + 1) * 512],
                        start=(ct == 0), stop=(ct == NP - 1))
                nc.vector.tensor_copy(o_sb[:, ch * 512:(ch + 1) * 512], pp[:])
            nc.sync.dma_start(out_d.ap()[tb * 128:(tb + 1) * 128, :], o_sb[:])

        # ---------------- Orchestration ----------------
        # Quarter 0 straight through; attention chunk c needs Q quarter c
        # and K/V quarters <= c, so chunk c overlaps projection quarter
        # c+1 and the output projection of earlier chunks.
        for u in p1_units(0):
            u()

        fill_by_chunk = {
            0: list(p1_units(1)),
            1: list(p1_units(2)) + [lambda tb=tb: proj_unit(tb)
                                    for tb in range(0, 4)],
            2: list(p1_units(3)),
            3: [lambda tb=tb: proj_unit(tb) for tb in range(4, 12)],
        }

        for cch in range(NCH):
            fill = fill_by_chunk[cch]
            nsteps = NP * (4 * cch + 4)
            quota = [0] * nsteps
            for i in range(len(fill)):
                quota[(i * nsteps) // len(fill)] += 1
            it = iter(range(nsteps))
            step_i = [0]

            def interleave():
                i = step_i[0]
                step_i[0] += 1
                for _ in range(quota[i] if i < nsteps else 0):
                    fill.pop(0)()

            for p in range(NP):
                attn_chunk_pair(p, cch, interleave)
            for u in fill:
                u()

        for tb in range(12, 16):
            proj_unit(tb)

        if dbg:
            nc.sync.dma_start(dbg["qTp0"].ap()[:], qTp[0][:])
            nc.sync.dma_start(dbg["kTp0"].ap()[:], kTp[0][:])
            nc.sync.dma_start(
                dbg["v_aug"].ap()[:],
                v_aug[:].rearrange("p j h d -> p (j h d)"))
            for m in range(NP):
                nc.sync.dma_start(dbg[f"yT{m}"].ap()[:], yT[m][:])

    nc.compile()
    return nc


def _get_nc():
    if "nc" not in _NC_CACHE:
        _register_ntff_hook()
        _NC_CACHE["nc"] = _build()
    return _NC_CACHE["nc"]


def kernel(x, w_attn, b_attn, w_proj, b_proj, _run_kwargs=None):
    import ml_dtypes
    from concourse.bass_utils import run_bass_kernel_spmd

    bf16 = ml_dtypes.bfloat16
    x = np.asarray(x, dtype=np.float32)
    w_attn = np.asarray(w_attn, dtype=np.float32)
    b_attn = np.asarray(b_attn, dtype=np.float32)
    w_proj = np.asarray(w_proj, dtype=np.float32)
    b_proj = np.asarray(b_proj, dtype=np.float32)

    nc = _get_nc()
    in_maps = []
    for core in range(NCORES):
        b, g = divmod(core, 2)
        cols = slice(g * CG, (g + 1) * CG)
        # xq[p, q, c, t] = x[b, q*512+t, c*128+p]
        xq = np.ascontiguousarray(
            x[b].reshape(NCH, 512, CT, 128).transpose(3, 0, 2, 1)
        ).reshape(128, -1)
        # wqk[p, s, c, n] = w_attn[c*128+p, s*C + g*CG + n]
        wqk = np.stack(
            [w_attn[:, cols], w_attn[:, C + g * CG: C + (g + 1) * CG]],
            axis=0).reshape(2, CT, 128, CG).transpose(2, 0, 1, 3)
        wv = w_attn[:, 2 * C + g * CG: 2 * C + (g + 1) * CG]
        bqk = np.concatenate(
            [b_attn[cols], b_attn[C + g * CG: C + (g + 1) * CG]])
        in_maps.append({
            "xq": xq.astype(bf16),
            "wqk": np.ascontiguousarray(wqk).reshape(128, -1).astype(bf16),
            "wv": np.ascontiguousarray(
                wv.reshape(CT, 128, CG).transpose(1, 0, 2)
            ).reshape(128, -1).astype(bf16),
            "wp": np.ascontiguousarray(
                w_proj[g * CG:(g + 1) * CG, :].reshape(NP, 128, C)
                .transpose(1, 0, 2)).reshape(128, -1).astype(bf16),
            "bqkT": np.ascontiguousarray(
                bqk.reshape(8, 128).T).astype(np.float32),
            "bv": np.ascontiguousarray(
                b_attn[2 * C + g * CG: 2 * C + (g + 1) * CG]).reshape(1, -1).astype(bf16),
        })

    res = run_bass_kernel_spmd(nc, in_maps, core_ids=list(range(NCORES)),
                               **(_run_kwargs or {}))
    out = np.empty((B, T, C), dtype=np.float32)
    for b in range(B):
        out[b] = (res.results[2 * b]["out"].astype(np.float32)
                  + res.results[2 * b + 1]["out"].astype(np.float32) + b_proj)
    if _run_kwargs:
        kernel.last_results = res
    return out
